# revision 27
# baseline (speedup 1.0000x reference)
"""Trainium2 Bass kernel for nn_Attention (B=2, S=2048, D=2048, H=16, hd=128).

Sharding: 2-way batch DP x 4-way head TP over 8 cores.
Core c: batch b = c//4, head-group g = c%4 (heads 4g..4g+4).

Per-core pipeline (single SPMD program, per-core behavior via input data only):
  Per s-quarter q (512 rows):
    Phase 1: QKV projections from pre-transposed x (x^T in HBM), RoPE applied
             to Q^T/K^T in [hd, S] layout. The hd axis of q/k weights is
             host-permuted (even indices first) so RoPE pairs become partition
             halves (i, 64+i); scores are invariant to a shared q/k hd-perm.
             All of wq/wk/wv/wo stay resident in SBUF after the first load.
    Phase 2: causal attention for q-tile q, all heads: scores computed
             TRANSPOSED (sT[k, q-tile] = K^T.T @ Q^T), mask on diag blocks,
             exp on ACT, row-sums via ones-matmul, PV matmul -> attn^T[hd, q],
             normalized by 1/l broadcast (K=1 ones matmul).
  AllGather attn^T shards within each batch group of 4 cores.
  Phase 3: out-proj slice: out[:, 512 cols of this group] from full attn^T.

All matmuls use f16 operands into f32 PSUM.
"""

import math
import sys

import numpy as np

for _p in ("/opt/trn_rl_repo",):
    if _p not in sys.path:
        sys.path.insert(0, _p)

import concourse.bass as bass
import concourse.mybir as mybir
from concourse import bacc
from concourse.tile import TileContext

B, S, D, H, HD = 2, 2048, 2048, 16, 128
NC_TOTAL = 8
TPG = 4                 # head-TP group size
HPC = H // TPG          # heads per core = 4
P = 128
NDC = D // P            # 16 contraction chunks
ST = 512                # s/q tile width
NST = S // ST           # 4

f32 = mybir.dt.float32
f32r = mybir.dt.float32r
f16 = mybir.dt.float16
AF = mybir.ActivationFunctionType
ALU = mybir.AluOpType

_NC_CACHE = {}


def build(sim_single_core: bool = False, null_kernel: bool = False,
          repeat: int = 1, hw_loop: bool = False) -> bass.Bass:
    """null_kernel=True: same I/O signature + collective, ~zero compute.
    Used to measure the axon dispatch floor for timing by difference.
    repeat=N: execute the whole kernel body N times back-to-back on-device
    (identical output). Used by test.py to amortize the per-dispatch axon
    RPC overhead out of the per-execution timing measurement."""
    nc = bacc.Bacc("TRN2", target_bir_lowering=False, debug=False,
                   num_devices=NC_TOTAL)

    xt = nc.declare_dram_parameter("xt", [D, S], f16, isOutput=False)
    wq_t = nc.declare_dram_parameter("wq_t", [D, HPC * HD], f16, isOutput=False)
    wk_t = nc.declare_dram_parameter("wk_t", [D, HPC * HD], f16, isOutput=False)
    wv_t = nc.declare_dram_parameter("wv_t", [D, HPC * HD], f16, isOutput=False)
    wo_t = nc.declare_dram_parameter("wo_t", [D, ST], f16, isOutput=False)
    cs2 = nc.declare_dram_parameter("cs2", [P, S], f32, isOutput=False)
    sn2 = nc.declare_dram_parameter("sn2", [P, S], f32, isOutput=False)
    mb = nc.declare_dram_parameter("mb", [P, P], f32, isOutput=False)
    out = nc.declare_dram_parameter("out", [S, ST], f32, isOutput=True)

    if null_kernel:
        with TileContext(nc) as tc:
            with (
                tc.tile_pool(name="sb", bufs=1) as sb,
                tc.tile_pool(name="dram", bufs=1, space="DRAM") as dpool,
            ):
                cc_in = dpool.tile([HPC * HD, ST], f16)
                cc_out = dpool.tile([D, ST], f16)
                t = sb.tile([P, ST], f16)
                nc.sync.dma_start(t[:], xt[0:P, 0:ST])
                nc.sync.dma_start(cc_in[0:P, :], t[:])
                nc.gpsimd.collective_compute(
                    "AllGather", ALU.bypass,
                    replica_groups=[[0, 1, 2, 3], [4, 5, 6, 7]],
                    ins=[cc_in[:]], outs=[cc_out[:]])
                t2 = sb.tile([P, ST], f32)
                nc.vector.tensor_copy(t2[:], t[:])
                nc.sync.dma_start(out[0:P, :], t2[:])
        nc.compile()
        return nc

    with TileContext(nc) as tc:
        with (
            tc.tile_pool(name="const", bufs=1) as cpool,
            tc.tile_pool(name="big", bufs=1) as big,
            tc.tile_pool(name="ps", bufs=1, space="PSUM") as ps,
            tc.tile_pool(name="dram", bufs=1, space="DRAM") as dpool,
        ):
            # ---- constants / persistent ----
            cs_sb = cpool.tile([P, S], f32)   # [cos; cos] stacked halves
            sn_sb = cpool.tile([P, S], f32)   # [sin; -sin] stacked halves
            mb_sb = cpool.tile([P, P], f32)   # one 128x128 causal triangle
            ones_col = cpool.tile([P, 1], f16)
            ones_row = cpool.tile([1, P], f32r)
            ones_f = cpool.tile([P, 1], f32)
            onesr_f = cpool.tile([1, P], f32)
            wo_sb = cpool.tile([P, NDC, ST], f16)
            wq_sb = cpool.tile([P, NDC, HPC * HD], f16)
            wk_sb = cpool.tile([P, NDC, HPC * HD], f16)
            wv_sb = cpool.tile([P, NDC, HPC * HD], f16)
            nc.vector.memset(ones_f[:], 1.0)
            nc.vector.memset(onesr_f[:], 1.0)
            nc.vector.tensor_copy(ones_col[:], ones_f[:])
            nc.vector.tensor_copy(ones_row[:], onesr_f[:])

            kt_all = big.tile([P, HPC, S], f16)           # K^T (rope'd, perm)
            qt_all = big.tile([P, HPC, S], f16)           # Q^T (rope'd, perm)
            v_all = big.tile([P, S // P, HPC * HD], f16)  # [s%128, s//128, h*hd]

            # Per-quarter collective buffers: AllGather_q launches right after
            # quarter q's attention; phase 3 for its s-tiles follows, all
            # overlapped with later quarters' compute.
            cc_in_q = [[dpool.tile([2 * HD, ST], f16, name=f"cc_in{j}_{p}")
                        for p in range(2)] for j in range(NST)]
            cc_out_q = [[dpool.tile([D // 2, ST], f16, name=f"cc_out{j}_{p}")
                         for p in range(2)] for j in range(NST)]

            with tc.tile_pool(name="p12", bufs=1) as p12:

                def rope_from_psum(dst, qk_ps, s0):
                    """RoPE in [hd, ST] layout; pairs are partitions (i, 64+i).
                    dst/qk_ps = [128, ST]; s0 = global s offset.
                    A = qk * [cos;cos]  (one full-width op).
                    U is built half-swapped straight from PSUM (mixed-space
                    ops may cross partition bases; SB+SB ops may not):
                    u[0:64] = qk[64:128] * (-sin), u[64:128] = qk[0:64] * sin.
                    dst = A + U  (one full-width op)."""
                    ssl = slice(s0, s0 + ST)
                    a_t = p12.tile([P, ST], f32, tag="rt", bufs=3, name="rt_a")
                    u_t = p12.tile([P, ST], f32, tag="rt", bufs=3, name="rt_u")
                    nc.vector.tensor_tensor(
                        a_t[:], qk_ps[:, :], cs_sb[:, ssl], ALU.mult)
                    nc.vector.tensor_tensor(
                        u_t[0:64, :], qk_ps[64:128, :], sn_sb[0:64, ssl],
                        ALU.mult)
                    nc.vector.tensor_tensor(
                        u_t[64:128, :], qk_ps[0:64, :], sn_sb[64:128, ssl],
                        ALU.mult)
                    nc.vector.tensor_tensor(
                        dst[:, :], a_t[:, :], u_t[:, :], ALU.add)

                def load_consts():
                    nc.sync.dma_start(cs_sb[:], cs2[:])
                    nc.sync.dma_start(sn_sb[:], sn2[:])
                    nc.sync.dma_start(mb_sb[:], mb[:])
                    for dg in range(4):
                        sl = slice(dg * 4, (dg + 1) * 4)
                        rsl = slice(dg * 4 * P, (dg + 1) * 4 * P)
                        nc.sync.dma_start(
                            wv_sb[:, sl, :],
                            wv_t[rsl, :].rearrange("(o p) f -> p o f", p=P))
                        nc.sync.dma_start(
                            wq_sb[:, sl, :],
                            wq_t[rsl, :].rearrange("(o p) f -> p o f", p=P))
                        nc.sync.dma_start(
                            wk_sb[:, sl, :],
                            wk_t[rsl, :].rearrange("(o p) f -> p o f", p=P))
                        nc.sync.dma_start(
                            wo_sb[:, sl, :],
                            wo_t[rsl, :].rearrange("(o p) f -> p o f", p=P))

                def emit_quarter(q, with_consts=False):
                    s0 = q * ST
                    # ---------- phase 1 (s-quarter q) ----------
                    xt_q = p12.tile([P, NDC, ST], f16, tag="xtq", bufs=2,
                                    name="xt_q")
                    for dg in range(4):
                        nc.sync.dma_start(
                            xt_q[:, dg * 4:(dg + 1) * 4, :],
                            xt[dg * 4 * P:(dg + 1) * 4 * P,
                               s0:s0 + ST].rearrange("(o p) s -> p o s", p=P))
                    if with_consts:
                        # consts are needed later than x/wv; load them after
                        # the first quarter's critical-path streams.
                        load_consts()

                    # V for the 4 s-chunks of this quarter (2 psum slots,
                    # 2 sc per pass)
                    for vs in range(2):
                        v_ps = [
                            ps.tile([P, HPC * HD], f32, tag="vps", bufs=2,
                                    name=f"vps_{vs}_{i}")
                            for i in range(2)
                        ]
                        for dc in range(NDC):
                            for i in range(2):
                                sc = vs * 2 + i
                                nc.tensor.matmul(
                                    v_ps[i][:],
                                    xt_q[:, dc, sc * P:(sc + 1) * P],
                                    wv_sb[:, dc, :],
                                    start=(dc == 0), stop=(dc == NDC - 1),
                                    skip_group_check=True,
                                )
                        for i in range(2):
                            nc.scalar.activation(
                                v_all[:, q * 4 + vs * 2 + i, :], v_ps[i][:],
                                AF.Copy)

                    # Q^T / K^T for this quarter with RoPE
                    for h in range(HPC):
                        qt_ps = ps.tile([P, ST], f32, tag="qk", bufs=3,
                                        name="qt_ps")
                        for dc in range(NDC):
                            nc.tensor.matmul(
                                qt_ps[:], wq_sb[:, dc, h * HD:(h + 1) * HD],
                                xt_q[:, dc, :],
                                start=(dc == 0), stop=(dc == NDC - 1),
                                skip_group_check=True,
                            )
                        rope_from_psum(qt_all[:, h, s0:s0 + ST], qt_ps, s0)
                        kt_ps = ps.tile([P, ST], f32, tag="qk", bufs=3,
                                        name="kt_ps")
                        for dc in range(NDC):
                            nc.tensor.matmul(
                                kt_ps[:], wk_sb[:, dc, h * HD:(h + 1) * HD],
                                xt_q[:, dc, :],
                                start=(dc == 0), stop=(dc == NDC - 1),
                                skip_group_check=True,
                            )
                        rope_from_psum(kt_all[:, h, s0:s0 + ST], kt_ps, s0)

                    # ---------- phase 2 (q-tile q, all heads) ----------
                    # Causal narrowing: for diagonal chunk d (keys at block
                    # d of this quarter), only query columns >= d*128 attend;
                    # score/exp/PV/tree all operate on [:, c0:] with
                    # c0 = d*128. The d==0 chunk is full width, so group
                    # base tiles are always valid over the full 512.
                    kcs = 4 * q + 4          # causal: key chunks 0..kcs-1
                    for h in range(HPC):
                        l_ps = ps.tile([1, ST], f32, tag="lob", bufs=2,
                                       name="l_ps")
                        o_ps = ps.tile([P, ST], f32, tag="lob", bufs=2,
                                       name="o_ps")
                        prev_pt = prev_c0 = None
                        grp_pt = None
                        for kc in range(kcs):
                            d = kc - 4 * q    # >= 0 on the diagonal quarter
                            c0 = d * P if d > 0 else 0
                            st_ps = ps.tile([P, ST], f32, tag="qk", bufs=3,
                                            name="st_ps")
                            nc.tensor.matmul(
                                st_ps[:, c0:],
                                kt_all[:, h, kc * P:(kc + 1) * P],
                                qt_all[:, h, s0 + c0:s0 + ST],
                                start=True, stop=True,
                                skip_group_check=True,
                            )
                            if d >= 0:   # diagonal 128-block: apply mask
                                nc.vector.tensor_tensor(
                                    st_ps[:, d * P:(d + 1) * P],
                                    st_ps[:, d * P:(d + 1) * P],
                                    mb_sb[:, :],
                                    ALU.add)
                            pt_sb = p12.tile([P, ST], f16, tag="pt", bufs=6,
                                             name="pt_sb")
                            nc.scalar.activation(
                                pt_sb[:, c0:], st_ps[:, c0:], AF.Exp)
                            # 4-way tree PT reduction, accumulated in place
                            # into the group-base tile; PE's ones-matmul runs
                            # on group sums only.
                            if kc % 2 == 0:
                                prev_pt, prev_c0 = pt_sb, c0
                            else:
                                nc.vector.tensor_tensor(
                                    prev_pt[:, c0:], prev_pt[:, c0:],
                                    pt_sb[:, c0:], ALU.add)
                                if kc % 4 == 1:
                                    grp_pt = prev_pt
                                else:
                                    nc.vector.tensor_tensor(
                                        grp_pt[:, prev_c0:],
                                        grp_pt[:, prev_c0:],
                                        prev_pt[:, prev_c0:], ALU.add)
                                    nc.tensor.matmul(
                                        l_ps[:], ones_col[:], grp_pt[:],
                                        start=(kc == 3), stop=(kc == kcs - 1),
                                        skip_group_check=True,
                                    )
                            nc.tensor.matmul(
                                o_ps[:, c0:],
                                v_all[:, kc, h * HD:(h + 1) * HD],
                                pt_sb[:, c0:],
                                start=(kc == 0), stop=(kc == kcs - 1),
                                skip_group_check=True,
                            )
                        recip = p12.tile([1, ST], f32r, tag="rcp", bufs=2,
                                         name="recip")
                        with nc.allow_low_precision(
                                reason="1/l rounded to f32r for bcast matmul"):
                            nc.vector.reciprocal(recip[:], l_ps[:])
                        bc_ps = ps.tile([P, ST], f32, tag="lob", bufs=2,
                                        name="bc_ps")
                        nc.tensor.matmul(
                            bc_ps[:], ones_row[:], recip[:],
                            start=True, stop=True, skip_group_check=True,
                        )
                        bc_sb = p12.tile([P, ST], f32, tag="bcs", bufs=2,
                                         name="bc_sb")
                        nc.scalar.activation(bc_sb[:], bc_ps[:], AF.Copy)
                        at_sb = p12.tile([P, ST], f16, tag="at", bufs=2,
                                         name="at_sb")
                        nc.vector.tensor_tensor(
                            at_sb[:], o_ps[:], bc_sb[:], ALU.mult)
                        nc.sync.dma_start(
                            cc_in_q[q][h // 2][(h % 2) * P:(h % 2 + 1) * P, :],
                            at_sb[:])

                        if not sim_single_core and h % 2 == 1:
                            nc.gpsimd.collective_compute(
                                "AllGather", ALU.bypass,
                                replica_groups=[[0, 1, 2, 3], [4, 5, 6, 7]],
                                ins=[cc_in_q[q][h // 2][:]],
                                outs=[cc_out_q[q][h // 2][:]])

                    # ---------- sim-mode collective stand-in ----------
                    if sim_single_core:
                        for pc in range(2):
                            for hh in range(2):
                                tmp = p12.tile([P, ST], f16, tag="cc", bufs=2,
                                               name="cc_tmp")
                                nc.sync.dma_start(
                                    tmp[:],
                                    cc_in_q[q][pc][hh * P:(hh + 1) * P, :])
                                nc.sync.dma_start(
                                    cc_out_q[q][pc][hh * P:(hh + 1) * P, :],
                                    tmp[:])
                            zz = p12.tile([P, ST], f16, tag="cc", bufs=2,
                                          name="zz")
                            nc.vector.memset(zz[:], 0.0)
                            for r in range(2 * HD, D // 2, P):
                                nc.sync.dma_start(
                                    cc_out_q[q][pc][r:r + P, :], zz[:])

                    for st in range(4 * q, 4 * q + 4):
                        c0 = (st % 4) * P
                        # a_sb axes: [s-part, pc(head-pair), r(core), i, col]
                        a_sb = p12.tile([P, 2, 4, 2, P], f16, tag="acc",
                                        bufs=3, name="a_sb")
                        for pc in range(2):
                            nc.sync.dma_start(
                                a_sb[:, pc, :, :, :],
                                cc_out_q[q][pc][:, c0:c0 + P].rearrange(
                                    "(r i p) f -> p r i f", p=P, i=2),
                            )
                        o3_ps = ps.tile([P, ST], f32, tag="o3", bufs=1,
                                        name="o3_ps")
                        # piece-0 chunks first: they only depend on the first
                        # AllGather of this quarter, so they can run while the
                        # second is still in flight.
                        idxs = [(r, pc, i)
                                for pc in range(2) for r in range(4)
                                for i in range(2)]
                        for n_i, (r, pc, i) in enumerate(idxs):
                            dc = 4 * r + 2 * pc + i
                            nc.tensor.matmul(
                                o3_ps[:], a_sb[:, pc, r, i, :],
                                wo_sb[:, dc, :],
                                start=(n_i == 0), stop=(n_i == NDC - 1),
                                skip_group_check=True,
                            )
                        o3_sb = p12.tile([P, ST], f32, tag="o3s", bufs=2,
                                         name="o3_sb")
                        nc.scalar.activation(o3_sb[:], o3_ps[:], AF.Copy)
                        nc.sync.dma_start(out[st * P:(st + 1) * P, :], o3_sb[:])

                if hw_loop and repeat > 1:
                    load_consts()
                    with tc.For_i(0, repeat):
                        for q in range(NST):
                            emit_quarter(q)
                else:
                    for rep in range(repeat):
                        for q in range(NST):
                            emit_quarter(
                                q, with_consts=(rep == 0 and q == 0))

    nc.compile()
    return nc


def _get_nc(sim_single_core: bool = False, repeat: int = 1,
            hw_loop: bool = False) -> bass.Bass:
    key = (bool(sim_single_core), repeat, bool(hw_loop))
    if key not in _NC_CACHE:
        _NC_CACHE[key] = build(sim_single_core, repeat=repeat, hw_loop=hw_loop)
    return _NC_CACHE[key]


def make_core_inputs(x, freqs_cos, freqs_sin, mask, w_in, w_out):
    """Host-side sharding/layout prep. Returns list of 8 per-core input dicts."""
    x = np.asarray(x, np.float32)
    freqs_cos = np.asarray(freqs_cos, np.float32)
    freqs_sin = np.asarray(freqs_sin, np.float32)
    mask = np.asarray(mask, np.float32)
    w_in = np.asarray(w_in, np.float32)
    w_out = np.asarray(w_out, np.float32)

    perm = np.concatenate([np.arange(0, HD, 2), np.arange(1, HD, 2)])
    cs2 = np.ascontiguousarray(
        np.vstack([freqs_cos.T, freqs_cos.T]))           # [128, S]
    sn2 = np.ascontiguousarray(
        np.vstack([-freqs_sin.T, freqs_sin.T]))          # [128, S]
    mb = np.ascontiguousarray(mask[:P, :P].T)            # [128, 128] triangle
    xt_b = [np.ascontiguousarray(x[b].T).astype(np.float16) for b in range(B)]
    wo_T = np.ascontiguousarray(w_out.T)                 # [D, D]

    scale = 1.0 / math.sqrt(HD)
    in_maps = []
    for c in range(NC_TOTAL):
        b, g = c // TPG, c % TPG
        heads = range(g * HPC, (g + 1) * HPC)
        wq = np.vstack([w_in[h * HD:(h + 1) * HD][perm] for h in heads]) * scale
        wk = np.vstack([w_in[D + h * HD:D + (h + 1) * HD][perm] for h in heads])
        wv = np.vstack([w_in[2 * D + h * HD:2 * D + (h + 1) * HD] for h in heads])
        in_maps.append({
            "xt": xt_b[b],
            "wq_t": np.ascontiguousarray(wq.T).astype(np.float16),
            "wk_t": np.ascontiguousarray(wk.T).astype(np.float16),
            "wv_t": np.ascontiguousarray(wv.T).astype(np.float16),
            "wo_t": np.ascontiguousarray(wo_T[:, g * ST:(g + 1) * ST]).astype(np.float16),
            "cs2": cs2,
            "sn2": sn2,
            "mb": mb,
        })
    return in_maps


def run_spmd(inputs: dict, trace: bool = False):
    """Compile+run on cores 0-7. Returns (full_output, BassKernelResults)."""
    from concourse.bass_utils import run_bass_kernel_spmd

    in_maps = make_core_inputs(**inputs)
    nc = _get_nc(False)
    res = run_bass_kernel_spmd(nc, in_maps, list(range(NC_TOTAL)), trace=trace)
    out_full = np.empty((B, S, D), np.float32)
    for c in range(NC_TOTAL):
        b, g = c // TPG, c % TPG
        out_full[b, :, g * ST:(g + 1) * ST] = res.results[c]["out"]
    return out_full, res


def kernel(x, freqs_cos, freqs_sin, mask, w_in, w_out):
    out, _ = run_spmd(
        dict(x=x, freqs_cos=freqs_cos, freqs_sin=freqs_sin, mask=mask,
             w_in=w_in, w_out=w_out))
    return out


# revision 32
# speedup vs baseline: 1.0789x; 1.0789x over previous
"""Trainium2 Bass kernel for nn_Attention (B=2, S=2048, D=2048, H=16, hd=128).

Sharding: 2-way batch DP x 4-way head TP over 8 cores.
Core c: batch b = c//4, head-group g = c%4 (heads 4g..4g+4).

Per-core pipeline (single SPMD program, per-core behavior via input data only):
  Per s-quarter q (512 rows):
    Phase 1: QKV projections from pre-transposed x (x^T in HBM), RoPE applied
             to Q^T/K^T in [hd, S] layout. The hd axis of q/k weights is
             host-permuted (even indices first) so RoPE pairs become partition
             halves (i, 64+i); scores are invariant to a shared q/k hd-perm.
             All of wq/wk/wv/wo stay resident in SBUF after the first load.
    Phase 2: causal attention for q-tile q, all heads: scores computed
             TRANSPOSED (sT[k, q-tile] = K^T.T @ Q^T), mask on diag blocks,
             exp on ACT, row-sums via ones-matmul, PV matmul -> attn^T[hd, q],
             normalized by 1/l broadcast (K=1 ones matmul).
  AllGather attn^T shards within each batch group of 4 cores.
  Phase 3: out-proj slice: out[:, 512 cols of this group] from full attn^T.

All matmuls use f16 operands into f32 PSUM.
"""

import math
import sys

import numpy as np

for _p in ("/opt/trn_rl_repo",):
    if _p not in sys.path:
        sys.path.insert(0, _p)

import concourse.bass as bass
import concourse.mybir as mybir
from concourse import bacc
from concourse.tile import TileContext

B, S, D, H, HD = 2, 2048, 2048, 16, 128
NC_TOTAL = 8
TPG = 4                 # head-TP group size
HPC = H // TPG          # heads per core = 4
P = 128
NDC = D // P            # 16 contraction chunks
ST = 512                # s/q tile width
NST = S // ST           # 4

f32 = mybir.dt.float32
f32r = mybir.dt.float32r
f16 = mybir.dt.float16
AF = mybir.ActivationFunctionType
ALU = mybir.AluOpType

_NC_CACHE = {}


def build(sim_single_core: bool = False, null_kernel: bool = False,
          repeat: int = 1, hw_loop: bool = False) -> bass.Bass:
    """null_kernel=True: same I/O signature + collective, ~zero compute.
    Used to measure the axon dispatch floor for timing by difference.
    repeat=N: execute the whole kernel body N times back-to-back on-device
    (identical output). Used by test.py to amortize the per-dispatch axon
    RPC overhead out of the per-execution timing measurement."""
    nc = bacc.Bacc("TRN2", target_bir_lowering=False, debug=False,
                   num_devices=NC_TOTAL)

    xt = nc.declare_dram_parameter("xt", [D, S], f16, isOutput=False)
    wq_t = nc.declare_dram_parameter("wq_t", [D, HPC * HD], f16, isOutput=False)
    wk_t = nc.declare_dram_parameter("wk_t", [D, HPC * HD], f16, isOutput=False)
    wv_t = nc.declare_dram_parameter("wv_t", [D, HPC * HD], f16, isOutput=False)
    wo_t = nc.declare_dram_parameter("wo_t", [D, ST], f16, isOutput=False)
    cs2 = nc.declare_dram_parameter("cs2", [P, S], f32, isOutput=False)
    sn2 = nc.declare_dram_parameter("sn2", [P, S], f32, isOutput=False)
    mb = nc.declare_dram_parameter("mb", [P, P], f32, isOutput=False)
    out = nc.declare_dram_parameter("out", [S, ST], f32, isOutput=True)

    if null_kernel:
        with TileContext(nc) as tc:
            with (
                tc.tile_pool(name="sb", bufs=1) as sb,
                tc.tile_pool(name="dram", bufs=1, space="DRAM") as dpool,
            ):
                cc_in = dpool.tile([HPC * HD, ST], f16)
                cc_out = dpool.tile([D, ST], f16)
                t = sb.tile([P, ST], f16)
                nc.sync.dma_start(t[:], xt[0:P, 0:ST])
                nc.sync.dma_start(cc_in[0:P, :], t[:])
                nc.gpsimd.collective_compute(
                    "AllGather", ALU.bypass,
                    replica_groups=[[0, 1, 2, 3], [4, 5, 6, 7]],
                    ins=[cc_in[:]], outs=[cc_out[:]])
                t2 = sb.tile([P, ST], f32)
                nc.vector.tensor_copy(t2[:], t[:])
                nc.sync.dma_start(out[0:P, :], t2[:])
        nc.compile()
        return nc

    with TileContext(nc) as tc:
        with (
            tc.tile_pool(name="const", bufs=1) as cpool,
            tc.tile_pool(name="big", bufs=1) as big,
            tc.tile_pool(name="ps", bufs=1, space="PSUM") as ps,
            tc.tile_pool(name="dram", bufs=1, space="DRAM") as dpool,
        ):
            # ---- constants / persistent ----
            cs_sb = cpool.tile([P, S], f32)   # [cos; cos] stacked halves
            sn_sb = cpool.tile([P, S], f32)   # [sin; -sin] stacked halves
            mb_sb = cpool.tile([P, P], f32)   # one 128x128 causal triangle
            ones_col = cpool.tile([P, 1], f16)
            ones_row = cpool.tile([1, P], f32r)
            ones_f = cpool.tile([P, 1], f32)
            onesr_f = cpool.tile([1, P], f32)
            wo_sb = cpool.tile([P, NDC, ST], f16)
            wq_sb = cpool.tile([P, NDC, HPC * HD], f16)
            wk_sb = cpool.tile([P, NDC, HPC * HD], f16)
            wv_sb = cpool.tile([P, NDC, HPC * HD], f16)
            nc.vector.memset(ones_f[:], 1.0)
            nc.vector.memset(onesr_f[:], 1.0)
            nc.vector.tensor_copy(ones_col[:], ones_f[:])
            nc.vector.tensor_copy(ones_row[:], onesr_f[:])

            kt_all = big.tile([P, HPC, S], f16)           # K^T (rope'd, perm)
            qt_all = big.tile([P, HPC, S], f16)           # Q^T (rope'd, perm)
            v_all = big.tile([P, S // P, HPC * HD], f16)  # [s%128, s//128, h*hd]

            # Per-quarter collective buffers: AllGather_q launches right after
            # quarter q's attention; phase 3 for its s-tiles follows, all
            # overlapped with later quarters' compute.
            cc_in_q = [[dpool.tile([2 * HD, ST], f16, name=f"cc_in{j}_{p}")
                        for p in range(2)] for j in range(NST)]
            cc_out_q = [[dpool.tile([D // 2, ST], f16, name=f"cc_out{j}_{p}")
                         for p in range(2)] for j in range(NST)]

            with tc.tile_pool(name="p12", bufs=1) as p12:

                def rope_from_psum(dst, qk_ps, s0):
                    """RoPE in [hd, ST] layout; pairs are partitions (i, 64+i).
                    dst/qk_ps = [128, ST]; s0 = global s offset.
                    A = qk * [cos;cos]  (one full-width op).
                    U is built half-swapped straight from PSUM (mixed-space
                    ops may cross partition bases; SB+SB ops may not):
                    u[0:64] = qk[64:128] * (-sin), u[64:128] = qk[0:64] * sin.
                    dst = A + U  (one full-width op)."""
                    ssl = slice(s0, s0 + ST)
                    a_t = p12.tile([P, ST], f32, tag="rt", bufs=3, name="rt_a")
                    u_t = p12.tile([P, ST], f32, tag="rt", bufs=3, name="rt_u")
                    nc.vector.tensor_tensor(
                        a_t[:], qk_ps[:, :], cs_sb[:, ssl], ALU.mult)
                    nc.vector.tensor_tensor(
                        u_t[0:64, :], qk_ps[64:128, :], sn_sb[0:64, ssl],
                        ALU.mult)
                    nc.vector.tensor_tensor(
                        u_t[64:128, :], qk_ps[0:64, :], sn_sb[64:128, ssl],
                        ALU.mult)
                    nc.vector.tensor_tensor(
                        dst[:, :], a_t[:, :], u_t[:, :], ALU.add)

                def load_consts():
                    nc.sync.dma_start(cs_sb[:], cs2[:])
                    nc.sync.dma_start(sn_sb[:], sn2[:])
                    nc.sync.dma_start(mb_sb[:], mb[:])
                    for dg in range(4):
                        sl = slice(dg * 4, (dg + 1) * 4)
                        rsl = slice(dg * 4 * P, (dg + 1) * 4 * P)
                        nc.sync.dma_start(
                            wv_sb[:, sl, :],
                            wv_t[rsl, :].rearrange("(o p) f -> p o f", p=P))
                        nc.sync.dma_start(
                            wq_sb[:, sl, :],
                            wq_t[rsl, :].rearrange("(o p) f -> p o f", p=P))
                        nc.sync.dma_start(
                            wk_sb[:, sl, :],
                            wk_t[rsl, :].rearrange("(o p) f -> p o f", p=P))
                        nc.sync.dma_start(
                            wo_sb[:, sl, :],
                            wo_t[rsl, :].rearrange("(o p) f -> p o f", p=P))

                def load_xt(q):
                    s0 = q * ST
                    t = p12.tile([P, NDC, ST], f16, tag="xtq", bufs=2,
                                 name="xt_q")
                    for dg in range(4):
                        nc.sync.dma_start(
                            t[:, dg * 4:(dg + 1) * 4, :],
                            xt[dg * 4 * P:(dg + 1) * 4 * P,
                               s0:s0 + ST].rearrange("(o p) s -> p o s", p=P))
                    return t

                def emit_quarter(q, xt_q, next_q=None, with_consts=False):
                    """next_q: prefetch that quarter's x^T right after this
                    quarter's last xt read, BEFORE phase-3's a_sb DMAs are
                    queued — the sync engine issues DMAs in emission order,
                    and a_sb waits on the AllGather, which would otherwise
                    stall the next quarter's input stream behind it.
                    Returns the prefetched tile (or None)."""
                    s0 = q * ST
                    # ---------- phase 1 (s-quarter q) ----------
                    if with_consts:
                        # consts are needed later than x/wv; load them after
                        # the first quarter's critical-path streams.
                        load_consts()

                    # V for the 4 s-chunks of this quarter (2 psum slots,
                    # 2 sc per pass)
                    for vs in range(2):
                        v_ps = [
                            ps.tile([P, HPC * HD], f32, tag="vps", bufs=2,
                                    name=f"vps_{vs}_{i}")
                            for i in range(2)
                        ]
                        for dc in range(NDC):
                            for i in range(2):
                                sc = vs * 2 + i
                                nc.tensor.matmul(
                                    v_ps[i][:],
                                    xt_q[:, dc, sc * P:(sc + 1) * P],
                                    wv_sb[:, dc, :],
                                    start=(dc == 0), stop=(dc == NDC - 1),
                                    skip_group_check=True,
                                )
                        for i in range(2):
                            nc.scalar.activation(
                                v_all[:, q * 4 + vs * 2 + i, :], v_ps[i][:],
                                AF.Copy)

                    # Q^T / K^T for this quarter with RoPE
                    for h in range(HPC):
                        qt_ps = ps.tile([P, ST], f32, tag="qk", bufs=3,
                                        name="qt_ps")
                        for dc in range(NDC):
                            nc.tensor.matmul(
                                qt_ps[:], wq_sb[:, dc, h * HD:(h + 1) * HD],
                                xt_q[:, dc, :],
                                start=(dc == 0), stop=(dc == NDC - 1),
                                skip_group_check=True,
                            )
                        rope_from_psum(qt_all[:, h, s0:s0 + ST], qt_ps, s0)
                        kt_ps = ps.tile([P, ST], f32, tag="qk", bufs=3,
                                        name="kt_ps")
                        for dc in range(NDC):
                            nc.tensor.matmul(
                                kt_ps[:], wk_sb[:, dc, h * HD:(h + 1) * HD],
                                xt_q[:, dc, :],
                                start=(dc == 0), stop=(dc == NDC - 1),
                                skip_group_check=True,
                            )
                        rope_from_psum(kt_all[:, h, s0:s0 + ST], kt_ps, s0)

                    # prefetch next quarter's x^T now: ahead of phase-3's
                    # collective-gated a_sb DMAs in the sync queue.
                    xt_next = load_xt(next_q) if next_q is not None else None

                    # ---------- phase 2 (q-tile q, all heads) ----------
                    # Causal narrowing: for diagonal chunk d (keys at block
                    # d of this quarter), only query columns >= d*128 attend;
                    # score/exp/PV/tree all operate on [:, c0:] with
                    # c0 = d*128. The d==0 chunk is full width, so group
                    # base tiles are always valid over the full 512.
                    kcs = 4 * q + 4          # causal: key chunks 0..kcs-1
                    for h in range(HPC):
                        l_ps = ps.tile([1, ST], f32, tag="lob", bufs=2,
                                       name="l_ps")
                        o_ps = ps.tile([P, ST], f32, tag="lob", bufs=2,
                                       name="o_ps")
                        prev_pt = prev_c0 = None
                        grp_pt = None
                        for kc in range(kcs):
                            d = kc - 4 * q    # >= 0 on the diagonal quarter
                            c0 = d * P if d > 0 else 0
                            st_ps = ps.tile([P, ST], f32, tag="qk", bufs=3,
                                            name="st_ps")
                            nc.tensor.matmul(
                                st_ps[:, c0:],
                                kt_all[:, h, kc * P:(kc + 1) * P],
                                qt_all[:, h, s0 + c0:s0 + ST],
                                start=True, stop=True,
                                skip_group_check=True,
                            )
                            if d >= 0:   # diagonal 128-block: apply mask
                                nc.vector.tensor_tensor(
                                    st_ps[:, d * P:(d + 1) * P],
                                    st_ps[:, d * P:(d + 1) * P],
                                    mb_sb[:, :],
                                    ALU.add)
                            pt_sb = p12.tile([P, ST], f16, tag="pt", bufs=6,
                                             name="pt_sb")
                            nc.scalar.activation(
                                pt_sb[:, c0:], st_ps[:, c0:], AF.Exp)
                            # 4-way tree PT reduction, accumulated in place
                            # into the group-base tile; PE's ones-matmul runs
                            # on group sums only.
                            if kc % 2 == 0:
                                prev_pt, prev_c0 = pt_sb, c0
                            else:
                                nc.vector.tensor_tensor(
                                    prev_pt[:, c0:], prev_pt[:, c0:],
                                    pt_sb[:, c0:], ALU.add)
                                if kc % 4 == 1:
                                    grp_pt = prev_pt
                                else:
                                    nc.vector.tensor_tensor(
                                        grp_pt[:, prev_c0:],
                                        grp_pt[:, prev_c0:],
                                        prev_pt[:, prev_c0:], ALU.add)
                                    nc.tensor.matmul(
                                        l_ps[:], ones_col[:], grp_pt[:],
                                        start=(kc == 3), stop=(kc == kcs - 1),
                                        skip_group_check=True,
                                    )
                            nc.tensor.matmul(
                                o_ps[:, c0:],
                                v_all[:, kc, h * HD:(h + 1) * HD],
                                pt_sb[:, c0:],
                                start=(kc == 0), stop=(kc == kcs - 1),
                                skip_group_check=True,
                            )
                        recip = p12.tile([1, ST], f32r, tag="rcp", bufs=2,
                                         name="recip")
                        with nc.allow_low_precision(
                                reason="1/l rounded to f32r for bcast matmul"):
                            nc.vector.reciprocal(recip[:], l_ps[:])
                        bc_ps = ps.tile([P, ST], f32, tag="lob", bufs=2,
                                        name="bc_ps")
                        nc.tensor.matmul(
                            bc_ps[:], ones_row[:], recip[:],
                            start=True, stop=True, skip_group_check=True,
                        )
                        bc_sb = p12.tile([P, ST], f32, tag="bcs", bufs=2,
                                         name="bc_sb")
                        nc.scalar.activation(bc_sb[:], bc_ps[:], AF.Copy)
                        at_sb = p12.tile([P, ST], f16, tag="at", bufs=2,
                                         name="at_sb")
                        nc.vector.tensor_tensor(
                            at_sb[:], o_ps[:], bc_sb[:], ALU.mult)
                        nc.sync.dma_start(
                            cc_in_q[q][h // 2][(h % 2) * P:(h % 2 + 1) * P, :],
                            at_sb[:])

                        if not sim_single_core and h % 2 == 1:
                            nc.gpsimd.collective_compute(
                                "AllGather", ALU.bypass,
                                replica_groups=[[0, 1, 2, 3], [4, 5, 6, 7]],
                                ins=[cc_in_q[q][h // 2][:]],
                                outs=[cc_out_q[q][h // 2][:]])

                    # ---------- sim-mode collective stand-in ----------
                    if sim_single_core:
                        for pc in range(2):
                            for hh in range(2):
                                tmp = p12.tile([P, ST], f16, tag="cc", bufs=2,
                                               name="cc_tmp")
                                nc.sync.dma_start(
                                    tmp[:],
                                    cc_in_q[q][pc][hh * P:(hh + 1) * P, :])
                                nc.sync.dma_start(
                                    cc_out_q[q][pc][hh * P:(hh + 1) * P, :],
                                    tmp[:])
                            zz = p12.tile([P, ST], f16, tag="cc", bufs=2,
                                          name="zz")
                            nc.vector.memset(zz[:], 0.0)
                            for r in range(2 * HD, D // 2, P):
                                nc.sync.dma_start(
                                    cc_out_q[q][pc][r:r + P, :], zz[:])

                    for st in range(4 * q, 4 * q + 4):
                        c0 = (st % 4) * P
                        # a_sb axes: [s-part, pc(head-pair), r(core), i, col]
                        a_sb = p12.tile([P, 2, 4, 2, P], f16, tag="acc",
                                        bufs=3, name="a_sb")
                        for pc in range(2):
                            nc.sync.dma_start(
                                a_sb[:, pc, :, :, :],
                                cc_out_q[q][pc][:, c0:c0 + P].rearrange(
                                    "(r i p) f -> p r i f", p=P, i=2),
                            )
                        o3_ps = ps.tile([P, ST], f32, tag="o3", bufs=1,
                                        name="o3_ps")
                        # piece-0 chunks first: they only depend on the first
                        # AllGather of this quarter, so they can run while the
                        # second is still in flight.
                        idxs = [(r, pc, i)
                                for pc in range(2) for r in range(4)
                                for i in range(2)]
                        for n_i, (r, pc, i) in enumerate(idxs):
                            dc = 4 * r + 2 * pc + i
                            nc.tensor.matmul(
                                o3_ps[:], a_sb[:, pc, r, i, :],
                                wo_sb[:, dc, :],
                                start=(n_i == 0), stop=(n_i == NDC - 1),
                                skip_group_check=True,
                            )
                        o3_sb = p12.tile([P, ST], f32, tag="o3s", bufs=2,
                                         name="o3_sb")
                        nc.scalar.activation(o3_sb[:], o3_ps[:], AF.Copy)
                        nc.sync.dma_start(out[st * P:(st + 1) * P, :], o3_sb[:])

                    return xt_next

                if hw_loop and repeat > 1:
                    load_consts()
                    with tc.For_i(0, repeat):
                        cur = load_xt(0)
                        for q in range(NST):
                            cur = emit_quarter(
                                q, cur, next_q=(q + 1) % NST if q < NST - 1
                                else None)
                else:
                    seq = [q for _ in range(repeat) for q in range(NST)]
                    cur = load_xt(seq[0])
                    for n, q in enumerate(seq):
                        nxt = seq[n + 1] if n + 1 < len(seq) else None
                        cur = emit_quarter(
                            q, cur, next_q=nxt,
                            with_consts=(n == 0))

    nc.compile()
    return nc


def _get_nc(sim_single_core: bool = False, repeat: int = 1,
            hw_loop: bool = False) -> bass.Bass:
    key = (bool(sim_single_core), repeat, bool(hw_loop))
    if key not in _NC_CACHE:
        _NC_CACHE[key] = build(sim_single_core, repeat=repeat, hw_loop=hw_loop)
    return _NC_CACHE[key]


def make_core_inputs(x, freqs_cos, freqs_sin, mask, w_in, w_out):
    """Host-side sharding/layout prep. Returns list of 8 per-core input dicts."""
    x = np.asarray(x, np.float32)
    freqs_cos = np.asarray(freqs_cos, np.float32)
    freqs_sin = np.asarray(freqs_sin, np.float32)
    mask = np.asarray(mask, np.float32)
    w_in = np.asarray(w_in, np.float32)
    w_out = np.asarray(w_out, np.float32)

    perm = np.concatenate([np.arange(0, HD, 2), np.arange(1, HD, 2)])
    cs2 = np.ascontiguousarray(
        np.vstack([freqs_cos.T, freqs_cos.T]))           # [128, S]
    sn2 = np.ascontiguousarray(
        np.vstack([-freqs_sin.T, freqs_sin.T]))          # [128, S]
    mb = np.ascontiguousarray(mask[:P, :P].T)            # [128, 128] triangle
    xt_b = [np.ascontiguousarray(x[b].T).astype(np.float16) for b in range(B)]
    wo_T = np.ascontiguousarray(w_out.T)                 # [D, D]

    scale = 1.0 / math.sqrt(HD)
    in_maps = []
    for c in range(NC_TOTAL):
        b, g = c // TPG, c % TPG
        heads = range(g * HPC, (g + 1) * HPC)
        wq = np.vstack([w_in[h * HD:(h + 1) * HD][perm] for h in heads]) * scale
        wk = np.vstack([w_in[D + h * HD:D + (h + 1) * HD][perm] for h in heads])
        wv = np.vstack([w_in[2 * D + h * HD:2 * D + (h + 1) * HD] for h in heads])
        in_maps.append({
            "xt": xt_b[b],
            "wq_t": np.ascontiguousarray(wq.T).astype(np.float16),
            "wk_t": np.ascontiguousarray(wk.T).astype(np.float16),
            "wv_t": np.ascontiguousarray(wv.T).astype(np.float16),
            "wo_t": np.ascontiguousarray(wo_T[:, g * ST:(g + 1) * ST]).astype(np.float16),
            "cs2": cs2,
            "sn2": sn2,
            "mb": mb,
        })
    return in_maps


def run_spmd(inputs: dict, trace: bool = False):
    """Compile+run on cores 0-7. Returns (full_output, BassKernelResults)."""
    from concourse.bass_utils import run_bass_kernel_spmd

    in_maps = make_core_inputs(**inputs)
    nc = _get_nc(False)
    res = run_bass_kernel_spmd(nc, in_maps, list(range(NC_TOTAL)), trace=trace)
    out_full = np.empty((B, S, D), np.float32)
    for c in range(NC_TOTAL):
        b, g = c // TPG, c % TPG
        out_full[b, :, g * ST:(g + 1) * ST] = res.results[c]["out"]
    return out_full, res


def kernel(x, freqs_cos, freqs_sin, mask, w_in, w_out):
    out, _ = run_spmd(
        dict(x=x, freqs_cos=freqs_cos, freqs_sin=freqs_sin, mask=mask,
             w_in=w_in, w_out=w_out))
    return out


# revision 33
# speedup vs baseline: 1.1154x; 1.0338x over previous
"""Trainium2 Bass kernel for nn_Attention (B=2, S=2048, D=2048, H=16, hd=128).

Sharding: 2-way batch DP x 4-way head TP over 8 cores.
Core c: batch b = c//4, head-group g = c%4 (heads 4g..4g+4).

Per-core pipeline (single SPMD program, per-core behavior via input data only):
  Per s-quarter q (512 rows):
    Phase 1: QKV projections from pre-transposed x (x^T in HBM), RoPE applied
             to Q^T/K^T in [hd, S] layout. The hd axis of q/k weights is
             host-permuted (even indices first) so RoPE pairs become partition
             halves (i, 64+i); scores are invariant to a shared q/k hd-perm.
             All of wq/wk/wv/wo stay resident in SBUF after the first load.
    Phase 2: causal attention for q-tile q, all heads: scores computed
             TRANSPOSED (sT[k, q-tile] = K^T.T @ Q^T), mask on diag blocks,
             exp on ACT, row-sums via ones-matmul, PV matmul -> attn^T[hd, q],
             normalized by 1/l broadcast (K=1 ones matmul).
  AllGather attn^T shards within each batch group of 4 cores.
  Phase 3: out-proj slice: out[:, 512 cols of this group] from full attn^T.

All matmuls use f16 operands into f32 PSUM.
"""

import math
import sys

import numpy as np

for _p in ("/opt/trn_rl_repo",):
    if _p not in sys.path:
        sys.path.insert(0, _p)

import concourse.bass as bass
import concourse.mybir as mybir
from concourse import bacc
from concourse.tile import TileContext

B, S, D, H, HD = 2, 2048, 2048, 16, 128
NC_TOTAL = 8
TPG = 4                 # head-TP group size
HPC = H // TPG          # heads per core = 4
P = 128
NDC = D // P            # 16 contraction chunks
ST = 512                # s/q tile width
NST = S // ST           # 4

f32 = mybir.dt.float32
f32r = mybir.dt.float32r
f16 = mybir.dt.float16
AF = mybir.ActivationFunctionType
ALU = mybir.AluOpType

_NC_CACHE = {}


def build(sim_single_core: bool = False, null_kernel: bool = False,
          repeat: int = 1, hw_loop: bool = False) -> bass.Bass:
    """null_kernel=True: same I/O signature + collective, ~zero compute.
    Used to measure the axon dispatch floor for timing by difference.
    repeat=N: execute the whole kernel body N times back-to-back on-device
    (identical output). Used by test.py to amortize the per-dispatch axon
    RPC overhead out of the per-execution timing measurement."""
    nc = bacc.Bacc("TRN2", target_bir_lowering=False, debug=False,
                   num_devices=NC_TOTAL)

    xt = nc.declare_dram_parameter("xt", [D, S], f16, isOutput=False)
    wq_t = nc.declare_dram_parameter("wq_t", [D, HPC * HD], f16, isOutput=False)
    wk_t = nc.declare_dram_parameter("wk_t", [D, HPC * HD], f16, isOutput=False)
    wv_t = nc.declare_dram_parameter("wv_t", [D, HPC * HD], f16, isOutput=False)
    wo_t = nc.declare_dram_parameter("wo_t", [D, ST], f16, isOutput=False)
    cs2 = nc.declare_dram_parameter("cs2", [P, S], f32, isOutput=False)
    sn2 = nc.declare_dram_parameter("sn2", [P, S], f32, isOutput=False)
    mb = nc.declare_dram_parameter("mb", [P, P], f32, isOutput=False)
    out = nc.declare_dram_parameter("out", [S, ST], f32, isOutput=True)

    if null_kernel:
        with TileContext(nc) as tc:
            with (
                tc.tile_pool(name="sb", bufs=1) as sb,
                tc.tile_pool(name="dram", bufs=1, space="DRAM") as dpool,
            ):
                cc_in = dpool.tile([HPC * HD, ST], f16)
                cc_out = dpool.tile([D, ST], f16)
                t = sb.tile([P, ST], f16)
                nc.sync.dma_start(t[:], xt[0:P, 0:ST])
                nc.sync.dma_start(cc_in[0:P, :], t[:])
                nc.gpsimd.collective_compute(
                    "AllGather", ALU.bypass,
                    replica_groups=[[0, 1, 2, 3], [4, 5, 6, 7]],
                    ins=[cc_in[:]], outs=[cc_out[:]])
                t2 = sb.tile([P, ST], f32)
                nc.vector.tensor_copy(t2[:], t[:])
                nc.sync.dma_start(out[0:P, :], t2[:])
        nc.compile()
        return nc

    with TileContext(nc) as tc:
        with (
            tc.tile_pool(name="const", bufs=1) as cpool,
            tc.tile_pool(name="big", bufs=1) as big,
            tc.tile_pool(name="ps", bufs=1, space="PSUM") as ps,
            tc.tile_pool(name="dram", bufs=1, space="DRAM") as dpool,
        ):
            # ---- constants / persistent ----
            cs_sb = cpool.tile([P, S], f32)   # [cos; cos] stacked halves
            sn_sb = cpool.tile([P, S], f32)   # [sin; -sin] stacked halves
            mb_sb = cpool.tile([P, P], f32)   # one 128x128 causal triangle
            ones_col = cpool.tile([P, 1], f16)
            ones_f = cpool.tile([P, 1], f32)
            wo_sb = cpool.tile([P, NDC, ST], f16)
            wq_sb = cpool.tile([P, NDC, HPC * HD], f16)
            wk_sb = cpool.tile([P, NDC, HPC * HD], f16)
            wv_sb = cpool.tile([P, NDC, HPC * HD], f16)
            nc.vector.memset(ones_f[:], 1.0)
            nc.vector.tensor_copy(ones_col[:], ones_f[:])

            kt_all = big.tile([P, HPC, S], f16)           # K^T (rope'd, perm)
            qt_all = big.tile([P, HPC, S], f16)           # Q^T (rope'd, perm)
            v_all = big.tile([P, S // P, HPC * HD], f16)  # [s%128, s//128, h*hd]

            # Per-quarter collective buffers: AllGather_q launches right after
            # quarter q's attention; phase 3 for its s-tiles follows, all
            # overlapped with later quarters' compute.
            cc_in_q = [[dpool.tile([2 * HD, ST], f16, name=f"cc_in{j}_{p}")
                        for p in range(2)] for j in range(NST)]
            cc_out_q = [[dpool.tile([D // 2, ST], f16, name=f"cc_out{j}_{p}")
                         for p in range(2)] for j in range(NST)]

            with tc.tile_pool(name="p12", bufs=1) as p12:

                def rope_from_psum(dst, qk_ps, s0):
                    """RoPE in [hd, ST] layout; pairs are partitions (i, 64+i).
                    dst/qk_ps = [128, ST]; s0 = global s offset.
                    A = qk * [cos;cos]  (one full-width op).
                    U is built half-swapped straight from PSUM (mixed-space
                    ops may cross partition bases; SB+SB ops may not):
                    u[0:64] = qk[64:128] * (-sin), u[64:128] = qk[0:64] * sin.
                    dst = A + U  (one full-width op)."""
                    ssl = slice(s0, s0 + ST)
                    a_t = p12.tile([P, ST], f32, tag="rt", bufs=3, name="rt_a")
                    u_t = p12.tile([P, ST], f32, tag="rt", bufs=3, name="rt_u")
                    nc.vector.tensor_tensor(
                        a_t[:], qk_ps[:, :], cs_sb[:, ssl], ALU.mult)
                    nc.vector.tensor_tensor(
                        u_t[0:64, :], qk_ps[64:128, :], sn_sb[0:64, ssl],
                        ALU.mult)
                    nc.vector.tensor_tensor(
                        u_t[64:128, :], qk_ps[0:64, :], sn_sb[64:128, ssl],
                        ALU.mult)
                    nc.vector.tensor_tensor(
                        dst[:, :], a_t[:, :], u_t[:, :], ALU.add)

                def load_consts():
                    nc.sync.dma_start(cs_sb[:], cs2[:])
                    nc.sync.dma_start(sn_sb[:], sn2[:])
                    nc.sync.dma_start(mb_sb[:], mb[:])
                    for dg in range(4):
                        sl = slice(dg * 4, (dg + 1) * 4)
                        rsl = slice(dg * 4 * P, (dg + 1) * 4 * P)
                        nc.sync.dma_start(
                            wv_sb[:, sl, :],
                            wv_t[rsl, :].rearrange("(o p) f -> p o f", p=P))
                        nc.sync.dma_start(
                            wq_sb[:, sl, :],
                            wq_t[rsl, :].rearrange("(o p) f -> p o f", p=P))
                        nc.sync.dma_start(
                            wk_sb[:, sl, :],
                            wk_t[rsl, :].rearrange("(o p) f -> p o f", p=P))
                        nc.sync.dma_start(
                            wo_sb[:, sl, :],
                            wo_t[rsl, :].rearrange("(o p) f -> p o f", p=P))

                def load_xt(q):
                    s0 = q * ST
                    t = p12.tile([P, NDC, ST], f16, tag="xtq", bufs=2,
                                 name="xt_q")
                    for dg in range(4):
                        nc.sync.dma_start(
                            t[:, dg * 4:(dg + 1) * 4, :],
                            xt[dg * 4 * P:(dg + 1) * 4 * P,
                               s0:s0 + ST].rearrange("(o p) s -> p o s", p=P))
                    return t

                def emit_quarter(q, xt_q, next_q=None, with_consts=False):
                    """next_q: prefetch that quarter's x^T right after this
                    quarter's last xt read, BEFORE phase-3's a_sb DMAs are
                    queued — the sync engine issues DMAs in emission order,
                    and a_sb waits on the AllGather, which would otherwise
                    stall the next quarter's input stream behind it.
                    Returns the prefetched tile (or None)."""
                    s0 = q * ST
                    # ---------- phase 1 (s-quarter q) ----------
                    if with_consts:
                        # consts are needed later than x/wv; load them after
                        # the first quarter's critical-path streams.
                        load_consts()

                    # V for the 4 s-chunks of this quarter (2 psum slots,
                    # 2 sc per pass)
                    for vs in range(2):
                        v_ps = [
                            ps.tile([P, HPC * HD], f32, tag="vps", bufs=2,
                                    name=f"vps_{vs}_{i}")
                            for i in range(2)
                        ]
                        for dc in range(NDC):
                            for i in range(2):
                                sc = vs * 2 + i
                                nc.tensor.matmul(
                                    v_ps[i][:],
                                    xt_q[:, dc, sc * P:(sc + 1) * P],
                                    wv_sb[:, dc, :],
                                    start=(dc == 0), stop=(dc == NDC - 1),
                                    skip_group_check=True,
                                )
                        for i in range(2):
                            nc.scalar.activation(
                                v_all[:, q * 4 + vs * 2 + i, :], v_ps[i][:],
                                AF.Copy)

                    # Q^T / K^T for this quarter with RoPE
                    for h in range(HPC):
                        qt_ps = ps.tile([P, ST], f32, tag="qk", bufs=3,
                                        name="qt_ps")
                        for dc in range(NDC):
                            nc.tensor.matmul(
                                qt_ps[:], wq_sb[:, dc, h * HD:(h + 1) * HD],
                                xt_q[:, dc, :],
                                start=(dc == 0), stop=(dc == NDC - 1),
                                skip_group_check=True,
                            )
                        rope_from_psum(qt_all[:, h, s0:s0 + ST], qt_ps, s0)
                        kt_ps = ps.tile([P, ST], f32, tag="qk", bufs=3,
                                        name="kt_ps")
                        for dc in range(NDC):
                            nc.tensor.matmul(
                                kt_ps[:], wk_sb[:, dc, h * HD:(h + 1) * HD],
                                xt_q[:, dc, :],
                                start=(dc == 0), stop=(dc == NDC - 1),
                                skip_group_check=True,
                            )
                        rope_from_psum(kt_all[:, h, s0:s0 + ST], kt_ps, s0)

                    # prefetch next quarter's x^T now: ahead of phase-3's
                    # collective-gated a_sb DMAs in the sync queue.
                    xt_next = load_xt(next_q) if next_q is not None else None

                    # ---------- phase 2 (q-tile q, all heads) ----------
                    # Causal narrowing: for diagonal chunk d (keys at block
                    # d of this quarter), only query columns >= d*128 attend;
                    # score/exp/PV/tree all operate on [:, c0:] with
                    # c0 = d*128. The d==0 chunk is full width, so group
                    # base tiles are always valid over the full 512.
                    kcs = 4 * q + 4          # causal: key chunks 0..kcs-1
                    for h in range(HPC):
                        l_ps = ps.tile([1, ST], f32, tag="lob", bufs=2,
                                       name="l_ps")
                        o_ps = ps.tile([P, ST], f32, tag="lob", bufs=2,
                                       name="o_ps")
                        prev_pt = prev_c0 = None
                        grp_pt = None
                        for kc in range(kcs):
                            d = kc - 4 * q    # >= 0 on the diagonal quarter
                            c0 = d * P if d > 0 else 0
                            st_ps = ps.tile([P, ST], f32, tag="qk", bufs=3,
                                            name="st_ps")
                            nc.tensor.matmul(
                                st_ps[:, c0:],
                                kt_all[:, h, kc * P:(kc + 1) * P],
                                qt_all[:, h, s0 + c0:s0 + ST],
                                start=True, stop=True,
                                skip_group_check=True,
                            )
                            if d >= 0:   # diagonal 128-block: apply mask
                                nc.vector.tensor_tensor(
                                    st_ps[:, d * P:(d + 1) * P],
                                    st_ps[:, d * P:(d + 1) * P],
                                    mb_sb[:, :],
                                    ALU.add)
                            pt_sb = p12.tile([P, ST], f16, tag="pt", bufs=6,
                                             name="pt_sb")
                            nc.scalar.activation(
                                pt_sb[:, c0:], st_ps[:, c0:], AF.Exp)
                            # 4-way tree PT reduction, accumulated in place
                            # into the group-base tile; PE's ones-matmul runs
                            # on group sums only.
                            if kc % 2 == 0:
                                prev_pt, prev_c0 = pt_sb, c0
                            else:
                                nc.vector.tensor_tensor(
                                    prev_pt[:, c0:], prev_pt[:, c0:],
                                    pt_sb[:, c0:], ALU.add)
                                if kc % 4 == 1:
                                    grp_pt = prev_pt
                                else:
                                    nc.vector.tensor_tensor(
                                        grp_pt[:, prev_c0:],
                                        grp_pt[:, prev_c0:],
                                        prev_pt[:, prev_c0:], ALU.add)
                                    nc.tensor.matmul(
                                        l_ps[:], ones_col[:], grp_pt[:],
                                        start=(kc == 3), stop=(kc == kcs - 1),
                                        skip_group_check=True,
                                    )
                            nc.tensor.matmul(
                                o_ps[:, c0:],
                                v_all[:, kc, h * HD:(h + 1) * HD],
                                pt_sb[:, c0:],
                                start=(kc == 0), stop=(kc == kcs - 1),
                                skip_group_check=True,
                            )
                        recip = p12.tile([1, ST], f32, tag="rcp", bufs=2,
                                         name="recip")
                        nc.vector.reciprocal(recip[:], l_ps[:])
                        bc_sb = p12.tile([P, ST], f32, tag="bcs", bufs=2,
                                         name="bc_sb")
                        nc.gpsimd.partition_broadcast(bc_sb[:], recip[:])
                        at_sb = p12.tile([P, ST], f16, tag="at", bufs=2,
                                         name="at_sb")
                        nc.vector.tensor_tensor(
                            at_sb[:], o_ps[:], bc_sb[:], ALU.mult)
                        nc.sync.dma_start(
                            cc_in_q[q][h // 2][(h % 2) * P:(h % 2 + 1) * P, :],
                            at_sb[:])

                        if not sim_single_core and h % 2 == 1:
                            nc.gpsimd.collective_compute(
                                "AllGather", ALU.bypass,
                                replica_groups=[[0, 1, 2, 3], [4, 5, 6, 7]],
                                ins=[cc_in_q[q][h // 2][:]],
                                outs=[cc_out_q[q][h // 2][:]])

                    # ---------- sim-mode collective stand-in ----------
                    if sim_single_core:
                        for pc in range(2):
                            for hh in range(2):
                                tmp = p12.tile([P, ST], f16, tag="cc", bufs=2,
                                               name="cc_tmp")
                                nc.sync.dma_start(
                                    tmp[:],
                                    cc_in_q[q][pc][hh * P:(hh + 1) * P, :])
                                nc.sync.dma_start(
                                    cc_out_q[q][pc][hh * P:(hh + 1) * P, :],
                                    tmp[:])
                            zz = p12.tile([P, ST], f16, tag="cc", bufs=2,
                                          name="zz")
                            nc.vector.memset(zz[:], 0.0)
                            for r in range(2 * HD, D // 2, P):
                                nc.sync.dma_start(
                                    cc_out_q[q][pc][r:r + P, :], zz[:])

                    for st in range(4 * q, 4 * q + 4):
                        c0 = (st % 4) * P
                        # a_sb axes: [s-part, pc(head-pair), r(core), i, col]
                        a_sb = p12.tile([P, 2, 4, 2, P], f16, tag="acc",
                                        bufs=3, name="a_sb")
                        for pc in range(2):
                            nc.sync.dma_start(
                                a_sb[:, pc, :, :, :],
                                cc_out_q[q][pc][:, c0:c0 + P].rearrange(
                                    "(r i p) f -> p r i f", p=P, i=2),
                            )
                        o3_ps = ps.tile([P, ST], f32, tag="o3", bufs=1,
                                        name="o3_ps")
                        # piece-0 chunks first: they only depend on the first
                        # AllGather of this quarter, so they can run while the
                        # second is still in flight.
                        idxs = [(r, pc, i)
                                for pc in range(2) for r in range(4)
                                for i in range(2)]
                        for n_i, (r, pc, i) in enumerate(idxs):
                            dc = 4 * r + 2 * pc + i
                            nc.tensor.matmul(
                                o3_ps[:], a_sb[:, pc, r, i, :],
                                wo_sb[:, dc, :],
                                start=(n_i == 0), stop=(n_i == NDC - 1),
                                skip_group_check=True,
                            )
                        o3_sb = p12.tile([P, ST], f32, tag="o3s", bufs=2,
                                         name="o3_sb")
                        nc.scalar.activation(o3_sb[:], o3_ps[:], AF.Copy)
                        nc.sync.dma_start(out[st * P:(st + 1) * P, :], o3_sb[:])

                    return xt_next

                if hw_loop and repeat > 1:
                    load_consts()
                    with tc.For_i(0, repeat):
                        cur = load_xt(0)
                        for q in range(NST):
                            cur = emit_quarter(
                                q, cur, next_q=(q + 1) % NST if q < NST - 1
                                else None)
                else:
                    seq = [q for _ in range(repeat) for q in range(NST)]
                    cur = load_xt(seq[0])
                    for n, q in enumerate(seq):
                        nxt = seq[n + 1] if n + 1 < len(seq) else None
                        cur = emit_quarter(
                            q, cur, next_q=nxt,
                            with_consts=(n == 0))

    nc.compile()
    return nc


def _get_nc(sim_single_core: bool = False, repeat: int = 1,
            hw_loop: bool = False) -> bass.Bass:
    key = (bool(sim_single_core), repeat, bool(hw_loop))
    if key not in _NC_CACHE:
        _NC_CACHE[key] = build(sim_single_core, repeat=repeat, hw_loop=hw_loop)
    return _NC_CACHE[key]


def make_core_inputs(x, freqs_cos, freqs_sin, mask, w_in, w_out):
    """Host-side sharding/layout prep. Returns list of 8 per-core input dicts."""
    x = np.asarray(x, np.float32)
    freqs_cos = np.asarray(freqs_cos, np.float32)
    freqs_sin = np.asarray(freqs_sin, np.float32)
    mask = np.asarray(mask, np.float32)
    w_in = np.asarray(w_in, np.float32)
    w_out = np.asarray(w_out, np.float32)

    perm = np.concatenate([np.arange(0, HD, 2), np.arange(1, HD, 2)])
    cs2 = np.ascontiguousarray(
        np.vstack([freqs_cos.T, freqs_cos.T]))           # [128, S]
    sn2 = np.ascontiguousarray(
        np.vstack([-freqs_sin.T, freqs_sin.T]))          # [128, S]
    mb = np.ascontiguousarray(mask[:P, :P].T)            # [128, 128] triangle
    xt_b = [np.ascontiguousarray(x[b].T).astype(np.float16) for b in range(B)]
    wo_T = np.ascontiguousarray(w_out.T)                 # [D, D]

    scale = 1.0 / math.sqrt(HD)
    in_maps = []
    for c in range(NC_TOTAL):
        b, g = c // TPG, c % TPG
        heads = range(g * HPC, (g + 1) * HPC)
        wq = np.vstack([w_in[h * HD:(h + 1) * HD][perm] for h in heads]) * scale
        wk = np.vstack([w_in[D + h * HD:D + (h + 1) * HD][perm] for h in heads])
        wv = np.vstack([w_in[2 * D + h * HD:2 * D + (h + 1) * HD] for h in heads])
        in_maps.append({
            "xt": xt_b[b],
            "wq_t": np.ascontiguousarray(wq.T).astype(np.float16),
            "wk_t": np.ascontiguousarray(wk.T).astype(np.float16),
            "wv_t": np.ascontiguousarray(wv.T).astype(np.float16),
            "wo_t": np.ascontiguousarray(wo_T[:, g * ST:(g + 1) * ST]).astype(np.float16),
            "cs2": cs2,
            "sn2": sn2,
            "mb": mb,
        })
    return in_maps


def run_spmd(inputs: dict, trace: bool = False):
    """Compile+run on cores 0-7. Returns (full_output, BassKernelResults)."""
    from concourse.bass_utils import run_bass_kernel_spmd

    in_maps = make_core_inputs(**inputs)
    nc = _get_nc(False)
    res = run_bass_kernel_spmd(nc, in_maps, list(range(NC_TOTAL)), trace=trace)
    out_full = np.empty((B, S, D), np.float32)
    for c in range(NC_TOTAL):
        b, g = c // TPG, c % TPG
        out_full[b, :, g * ST:(g + 1) * ST] = res.results[c]["out"]
    return out_full, res


def kernel(x, freqs_cos, freqs_sin, mask, w_in, w_out):
    out, _ = run_spmd(
        dict(x=x, freqs_cos=freqs_cos, freqs_sin=freqs_sin, mask=mask,
             w_in=w_in, w_out=w_out))
    return out
